# revision 20
# baseline (speedup 1.0000x reference)
"""Trainium2 Bass kernel for nn_MultiHeadAttention_5076651344076.

Reference computation (B=4, S=1024, SIZE=768, H=12, HD=64):
    q = split_heads(x @ Wq + bq); k = ...; v = ...
    scores = (q @ k^T + 40*noise) / 8 + mask
    probs  = softmax(scores); ctx = probs @ v
    h = LN1(ctx @ Wd + bd + x); out = LN2(h + x)

Sharding: 8 cores = (batch b, seq half). Each core computes 512 query rows
against the full 1024-key sequence of its batch. To keep the program
SPMD-static, the host rolls each core's x rows (and noise columns / mask)
so the query rows are always tokens 0..511.

Math folding used on-device:
    logits = qk/8 + 5*noise + mask
           = 5*((q/40)@k^T + noise) + mask
    Wq,bq are host-prescaled by 1/40 so the scores matmul directly yields
    qk/40.  exp(mask) is folded into V' rows (and the fused sum-of-exp
    column), which is exactly softmax with the additive key mask.
"""

import os
import threading

import numpy as np

import concourse.bass as bass
import concourse.mybir as mybir
import concourse.tile as tile
from concourse import bacc
from concourse.masks import make_identity

F32 = mybir.dt.float32
F32R = mybir.dt.float32r
F16 = mybir.dt.float16

B, S, D, H, HD = 4, 1024, 768, 12, 64
NCORES = 8
SQ = 512            # query rows per core
QT = SQ // 128      # 4 query tiles
KT = S // 128       # 8 key token tiles
FC = D // 128       # 6 feature chunks
NPAIR = H // 2      # 6 head pairs
EXP_SCALE = 5.0
EXP_BIAS = -25.0
EPS1, EPS2 = 1e-12, 1e-5

Identity = mybir.ActivationFunctionType.Identity
Copy = mybir.ActivationFunctionType.Copy
Exp = mybir.ActivationFunctionType.Exp
Sqrt = mybir.ActivationFunctionType.Sqrt
ADD = mybir.AluOpType.add
SUB = mybir.AluOpType.subtract
MUL = mybir.AluOpType.mult


def build_program():
    nc = bacc.Bacc(trn_type="TRN2", num_devices=NCORES)

    x = nc.dram_tensor("x", [S, D], F32, kind="ExternalInput")
    noise = nc.dram_tensor("noise", [H, SQ, S], F32, kind="ExternalInput")
    mask = nc.dram_tensor("mask", [S], F32, kind="ExternalInput")
    wq = nc.dram_tensor("wq", [D, D], F32, kind="ExternalInput")
    wk = nc.dram_tensor("wk", [D, D], F32, kind="ExternalInput")
    wv = nc.dram_tensor("wv", [D, D], F32, kind="ExternalInput")
    wd = nc.dram_tensor("wd", [D, D], F32, kind="ExternalInput")
    bq = nc.dram_tensor("bq", [D], F32, kind="ExternalInput")
    bk = nc.dram_tensor("bk", [D], F32, kind="ExternalInput")
    bv = nc.dram_tensor("bv", [D], F32, kind="ExternalInput")
    bd = nc.dram_tensor("bd", [D], F32, kind="ExternalInput")
    g1 = nc.dram_tensor("g1", [D], F32, kind="ExternalInput")
    b1 = nc.dram_tensor("b1", [D], F32, kind="ExternalInput")
    g2 = nc.dram_tensor("g2", [D], F32, kind="ExternalInput")
    b2 = nc.dram_tensor("b2", [D], F32, kind="ExternalInput")
    out = nc.dram_tensor("out", [SQ, D], F32, kind="ExternalOutput")

    with tile.TileContext(nc) as tc:
        emit(nc, tc, x, noise, mask, wq, wk, wv, wd,
             bq, bk, bv, bd, g1, b1, g2, b2, out)
    nc.finalize()
    return nc


def emit(nc, tc, x, noise, mask, wq, wk, wv, wd,
         bq, bk, bv, bd, g1, b1, g2, b2, out):
    from contextlib import ExitStack

    ctx = ExitStack()
    with ctx:
        const = ctx.enter_context(tc.tile_pool(name="const", bufs=1))

        # ---- constants / broadcasts -------------------------------------
        ident = const.tile([128, 128], F32)
        make_identity(nc, ident)

        # all-ones rows; row 64 used as K=1 stationary for the recip bcast
        ones65f = const.tile([65, 128], F32)
        nc.vector.memset(ones65f, 1.0)
        ones65 = const.tile([65, 128], F32R)
        nc.scalar.copy(ones65, ones65f)

        cexpb = const.tile([128, 1], F32)   # exp bias
        nc.vector.memset(cexpb, EXP_BIAS)
        ceps1 = const.tile([128, 1], F32)
        nc.vector.memset(ceps1, EPS1)
        ceps2 = const.tile([128, 1], F32)
        nc.vector.memset(ceps2, EPS2)
        eps_tiles = {EPS1: ceps1, EPS2: ceps2}

        bqc = const.tile([128, FC], F32)   # bq (prescaled) as per-partition cols
        bkc = const.tile([128, FC], F32)
        nc.gpsimd.dma_start(out=bqc, in_=bq.ap().rearrange("(c p) -> p c", p=128))
        nc.gpsimd.dma_start(out=bkc, in_=bk.ap().rearrange("(c p) -> p c", p=128))

        mk = const.tile([128, KT], F32)
        nc.gpsimd.dma_start(out=mk, in_=mask.ap().rearrange("(t p) -> p t", p=128))
        emk = const.tile([128, KT], F32)
        nc.scalar.activation(emk, mk, Exp)

        bvb = const.tile([128, D], F32)    # row broadcasts across partitions
        g1b = const.tile([128, D], F32)
        b1b = const.tile([128, D], F32)
        g2b = const.tile([128, D], F32)
        b2b = const.tile([128, D], F32)
        bdb = const.tile([128, D], F32)
        for dst, src in ((bvb, bv), (g1b, g1), (b1b, b1),
                         (g2b, g2), (b2b, b2), (bdb, bd)):
            nc.gpsimd.dma_start(out=dst, in_=src.ap().partition_broadcast(128))

        # persistent outputs of the projection phase
        pqkv = ctx.enter_context(tc.tile_pool(name="pqkv", bufs=1))
        qts = pqkv.tile([128, FC, SQ], F32R)   # q^T/40 feature-major
        kts = pqkv.tile([128, FC, S], F32R)    # k^T feature-major
        vaug = pqkv.tile([128, KT, H, 65], F16)  # [(v+bv)*emk | emk] token-major
        bdx = const.tile([128, QT, D], F32)   # bd + x  (dense bias + residual)
        b1x = const.tile([128, QT, D], F32)   # b1 + x  (LN1 bias + residual)

        # ---- phase A/B: load x, x^T, projections (transient pools) ------
        ab = ExitStack()
        px = ab.enter_context(tc.tile_pool(name="px", bufs=1))
        pxt = ab.enter_context(tc.tile_pool(name="pxt", bufs=1))
        pw = ab.enter_context(tc.tile_pool(name="pw", bufs=FC + 3))
        pproj = ab.enter_context(tc.tile_pool(name="pproj", bufs=3, space="PSUM"))
        pvtmp = ab.enter_context(tc.tile_pool(name="pvtmp", bufs=3))

        xsb = px.tile([128, KT, D], F32)   # xsb[p, t, d] = x[t*128+p, d]
        nc.scalar.dma_start(out=xsb, in_=x.ap().rearrange("(t p) d -> p t d", p=128))

        for t in range(QT):
            nc.vector.tensor_tensor(bdx[:, t, :], xsb[:, t, :], bdb, ADD)
            nc.vector.tensor_tensor(b1x[:, t, :], xsb[:, t, :], b1b, ADD)

        xt = pxt.tile([128, FC, S], F32R)   # xt[p, c, t] = x[t, c*128+p]
        with tc.tile_pool(name="pstr", bufs=4, space="PSUM") as pstr:
            for c in range(FC):
                for t in range(KT):
                    ps = pstr.tile([128, 128], F32, name=f"tr_{c}_{t}", tag="tr")
                    nc.tensor.transpose(ps, xsb[:, t, c * 128:(c + 1) * 128], ident)
                    nc.scalar.copy(xt[:, c, t * 128:(t + 1) * 128], ps)

        for wsrc, dst, bias_c, ntok in ((wq, qts, bqc, SQ), (wk, kts, bkc, S)):
            wt = [pw.tile([128, D], F32R, name=f"w_{i}", tag="w") for i in range(FC)]
            for i in range(FC):
                nc.gpsimd.dma_start(out=wt[i], in_=wsrc.ap()[i * 128:(i + 1) * 128, :])
            for fo in range(FC):
                for th in range(ntok // 512):
                    ps = pproj.tile([128, 512], F32, name=f"pp_{fo}_{th}", tag="pp")
                    for kc in range(FC):
                        nc.tensor.matmul(
                            ps, wt[kc][:, fo * 128:(fo + 1) * 128],
                            xt[:, kc, th * 512:(th + 1) * 512],
                            start=(kc == 0), stop=(kc == FC - 1))
                    nc.scalar.activation(dst[:, fo, th * 512:(th + 1) * 512], ps,
                                         Identity, bias=bias_c[:, fo:fo + 1])

        # V token-major, fused bias + mask-exp scale, cast to fp16
        wt = [pw.tile([128, D], F32R, name=f"wv_{i}", tag="w") for i in range(FC)]
        for i in range(FC):
            nc.gpsimd.dma_start(out=wt[i], in_=wv.ap()[i * 128:(i + 1) * 128, :])
        for t in range(KT):
            for f0, fw in ((0, 512), (512, 256)):
                ps = pproj.tile([128, 512], F32, name=f"pv_{t}_{f0}", tag="pp")
                for kc in range(FC):
                    nc.tensor.matmul(
                        ps[:, :fw], xt[:, kc, t * 128:(t + 1) * 128],
                        wt[kc][:, f0:f0 + fw],
                        start=(kc == 0), stop=(kc == FC - 1))
                vt = pvtmp.tile([128, 512], F32, name=f"vt_{t}_{f0}", tag="vt")
                nc.vector.tensor_tensor(vt[:, :fw], ps[:, :fw], bvb[:, f0:f0 + fw], ADD)
                nc.vector.tensor_scalar(
                    out=vaug[:, t, f0 // 64:(f0 + fw) // 64, 0:64],
                    in0=vt[:, :fw].rearrange("p (h d) -> p h d", d=64),
                    scalar1=emk[:, t:t + 1], scalar2=None,
                    op0=MUL)
        for h in range(H):
            nc.scalar.copy(vaug[:, :, h, 64], emk)
        ab.close()

        # ---- attention ---------------------------------------------------
        pct = ctx.enter_context(tc.tile_pool(name="pct", bufs=1))
        cts = pct.tile([128, NPAIR, SQ], F32R)   # normalized ctx^T, head pairs

        att = ExitStack()
        patt = att.enter_context(tc.tile_pool(name="patt", bufs=3))
        ppt = att.enter_context(tc.tile_pool(name="ppt", bufs=2))
        psc = att.enter_context(tc.tile_pool(name="psc", bufs=1, space="PSUM"))
        psct = att.enter_context(tc.tile_pool(name="psct", bufs=2, space="PSUM"))
        psbc = att.enter_context(tc.tile_pool(name="psbc", bufs=2, space="PSUM"))
        pnrm = att.enter_context(tc.tile_pool(name="pnrm", bufs=2))

        for hp in range(NPAIR):
            pt = [ppt.tile([128, KT, SQ], F16, name=f"pt{i}_{hp}", tag=f"pt{i}")
                  for i in range(2)]
            for qt in range(QT):
                pss = [psc.tile([128, S], F32, name=f"ps_{hp}_{qt}_{i}", tag=f"ps{i}")
                       for i in range(2)]
                qsl = slice(qt * 128, (qt + 1) * 128)
                # 2-head row-packed fp32r scores: psum = (q/40) @ k^T
                for kh in range(2):
                    ksl = slice(kh * 512, (kh + 1) * 512)
                    for i in range(2):
                        hsl = slice(i * 64, (i + 1) * 64)
                        nc.tensor.matmul(
                            pss[i][:, ksl], qts[hsl, hp, qsl],
                            kts[hsl, hp, ksl],
                            start=True, stop=True, tile_position=(i * 64, 0))
                for i in range(2):
                    h = 2 * hp + i
                    nt = patt.tile([128, S], F32, name=f"nt_{hp}_{qt}_{i}", tag="nt")
                    nc.scalar.dma_start(out=nt, in_=noise.ap()[h, qsl, :])
                    for kh in range(2):
                        ksl = slice(kh * 512, (kh + 1) * 512)
                        lt = patt.tile([128, 512], F32,
                                       name=f"lt_{hp}_{qt}_{i}_{kh}", tag="lt")
                        nc.vector.tensor_tensor(lt, pss[i][:, ksl], nt[:, ksl], ADD)
                        et = patt.tile([128, 512], F16,
                                       name=f"et_{hp}_{qt}_{i}_{kh}", tag="et")
                        nc.scalar.activation(et, lt, Exp,
                                             bias=cexpb, scale=EXP_SCALE)
                        for j in range(4):
                            nc.sync.dma_start(
                                out=pt[i][:, kh * 4 + j, qsl],
                                in_=et[:, j * 128:(j + 1) * 128], transpose=True)

            # ctx per head: psum rows 0:64 = V'^T @ P^T, row 64 = sum-of-exp
            for i in range(2):
                h = 2 * hp + i
                pc = psct.tile([65, 512], F32, name=f"pc_{hp}_{i}", tag="pc")
                for t in range(KT):
                    nc.tensor.matmul(pc, vaug[:, t, h, 0:65], pt[i][:, t, :],
                                     start=(t == 0), stop=(t == KT - 1))
                # reciprocal of sumexp row, broadcast to 64 partitions via PE
                rt = pnrm.tile([65, 512], F32R, name=f"rt_{hp}_{i}", tag="rt")
                with nc.allow_low_precision(reason="f32r recip feeds f32r matmul"):
                    nc.vector.reciprocal(rt[64:65, :], pc[64:65, :])
                pb = psbc.tile([64, 512], F32, name=f"pb_{hp}_{i}", tag="pb")
                nc.tensor.matmul(pb, ones65[64:65, 0:64], rt[64:65, :],
                                 start=True, stop=True, tile_position=(64, 0))
                rb = pnrm.tile([64, 512], F32, name=f"rb_{hp}_{i}", tag="rb")
                nc.scalar.copy(rb, pb)
                if i == 0:
                    nc.vector.tensor_tensor(cts[0:64, hp, :], pc[0:64, :], rb, MUL)
                else:
                    co = pnrm.tile([64, 512], F32, name=f"co_{hp}", tag="co")
                    nc.vector.tensor_tensor(co, pc[0:64, :], rb, MUL)
                    # partition shift 0:64 -> 64:128 (only DMA can do this)
                    nc.gpsimd.dma_start(out=cts[64:128, hp, :], in_=co)
        att.close()

        # ---- dense + residual + two LayerNorms --------------------------
        pwd = ctx.enter_context(tc.tile_pool(name="pwd", bufs=1))
        wdt = [pwd.tile([128, D], F32R, name=f"wd_{i}", tag=f"wd_{i}") for i in range(FC)]
        for i in range(FC):
            nc.gpsimd.dma_start(out=wdt[i], in_=wd.ap()[i * 128:(i + 1) * 128, :])

        pd = ctx.enter_context(tc.tile_pool(name="pd", bufs=3))
        psd = ctx.enter_context(tc.tile_pool(name="psd", bufs=3, space="PSUM"))
        pst = ctx.enter_context(tc.tile_pool(name="pst", bufs=4))

        for t in range(QT):
            qsl = slice(t * 128, (t + 1) * 128)
            t1 = pd.tile([128, D], F32, name=f"t1_{t}", tag="t1")
            for f0, fw in ((0, 512), (512, 256)):
                ps = psd.tile([128, 512], F32, name=f"pd_{t}_{f0}", tag="pd")
                for dc in range(FC):
                    nc.tensor.matmul(ps[:, :fw], cts[:, dc, qsl],
                                     wdt[dc][:, f0:f0 + fw],
                                     start=(dc == 0), stop=(dc == FC - 1))
                nc.vector.tensor_tensor(t1[:, f0:f0 + fw], ps[:, :fw],
                                        bdx[:, t, f0:f0 + fw], ADD)

            u = pd.tile([128, D], F32, name=f"u_{t}", tag="u")
            layernorm(nc, pst, t1, u, eps_tiles[EPS1], t)
            nc.vector.tensor_tensor(u, u, g1b, MUL)
            nc.vector.tensor_tensor(u, u, b1x[:, t, :], ADD)

            v = pd.tile([128, D], F32, name=f"v_{t}", tag="v")
            layernorm(nc, pst, u, v, eps_tiles[EPS2], t + 4)
            nc.vector.tensor_tensor(v, v, g2b, MUL)
            nc.vector.tensor_tensor(v, v, b2b, ADD)
            nc.gpsimd.dma_start(out=out.ap()[qsl, :], in_=v)


def layernorm(nc, pool, src, dst, eps_tile, uid):
    """dst = (src - mean) * rsqrt(var + eps), stats along the 768 free dim."""
    st = pool.tile([128, 3, 6], F32, name=f"st_{uid}", tag="st")
    for sg in range(3):
        nc.vector.bn_stats(st[:, sg, :], src[:, sg * 256:(sg + 1) * 256])
    mv = pool.tile([128, 2], F32, name=f"mv_{uid}", tag="mv")
    nc.vector.bn_aggr(mv, st)
    sd = pool.tile([128, 1], F32, name=f"sd_{uid}", tag="sd")
    nc.scalar.activation(sd, mv[:, 1:2], Sqrt, bias=eps_tile)
    rs = pool.tile([128, 1], F32, name=f"rs_{uid}", tag="rs")
    nc.vector.reciprocal(rs, sd)
    nc.vector.tensor_scalar(out=dst, in0=src, scalar1=mv[:, 0:1], scalar2=rs,
                            op0=SUB, op1=MUL)


_lock = threading.Lock()
_cache = {}


def get_program():
    with _lock:
        if "nc" not in _cache:
            _cache["nc"] = build_program()
        return _cache["nc"]


def make_in_maps(inputs):
    x = np.ascontiguousarray(np.asarray(inputs["input_tensor"], dtype=np.float32))
    am = np.asarray(inputs["attention_mask"], dtype=np.float32)
    nz = np.asarray(inputs["noise"], dtype=np.float32)
    shared = {
        "wq": np.ascontiguousarray(np.asarray(inputs["Wq"], np.float32) / 40.0),
        "bq": np.ascontiguousarray(np.asarray(inputs["bq"], np.float32) / 40.0),
        "wk": np.ascontiguousarray(np.asarray(inputs["Wk"], np.float32)),
        "bk": np.ascontiguousarray(np.asarray(inputs["bk"], np.float32)),
        "wv": np.ascontiguousarray(np.asarray(inputs["Wv"], np.float32)),
        "bv": np.ascontiguousarray(np.asarray(inputs["bv"], np.float32)),
        "wd": np.ascontiguousarray(np.asarray(inputs["Wd"], np.float32)),
        "bd": np.ascontiguousarray(np.asarray(inputs["bd"], np.float32)),
        "g1": np.ascontiguousarray(np.asarray(inputs["g1"], np.float32)),
        "b1": np.ascontiguousarray(np.asarray(inputs["b1"], np.float32)),
        "g2": np.ascontiguousarray(np.asarray(inputs["g2"], np.float32)),
        "b2": np.ascontiguousarray(np.asarray(inputs["b2"], np.float32)),
    }
    in_maps = []
    for c in range(NCORES):
        b, half = c // 2, c % 2
        qoff = half * SQ
        m = dict(shared)
        m["x"] = np.ascontiguousarray(np.roll(x[b], -qoff, axis=0))
        m["noise"] = np.ascontiguousarray(
            np.roll(nz[b, :, qoff:qoff + SQ, :], -qoff, axis=-1))
        m["mask"] = np.ascontiguousarray(np.roll(am[b, 0, 0], -qoff))
        in_maps.append(m)
    return in_maps


def kernel(**inputs):
    from concourse.bass_utils import run_bass_kernel_spmd

    nc = get_program()
    in_maps = make_in_maps(inputs)
    res = run_bass_kernel_spmd(
        nc, in_maps, core_ids=list(range(NCORES)),
        trace=bool(int(os.environ.get("KERNEL_TRACE", "0"))))
    kernel.last_results = res
    outp = np.empty((B, S, D), dtype=np.float32)
    for c in range(NCORES):
        b, half = c // 2, c % 2
        outp[b, half * SQ:(half + 1) * SQ, :] = res.results[c]["out"]
    return outp


# revision 23
# speedup vs baseline: 2.0986x; 2.0986x over previous
"""Trainium2 Bass kernel for nn_MultiHeadAttention_5076651344076.

Reference computation (B=4, S=1024, SIZE=768, H=12, HD=64):
    q = split_heads(x @ Wq + bq); k = ...; v = ...
    scores = (q @ k^T + 40*noise) / 8 + mask
    probs  = softmax(scores); ctx = probs @ v
    h = LN1(ctx @ Wd + bd + x); out = LN2(h + x)

Sharding: 8 cores = (batch b, seq half). Each core computes 512 query rows
against the full 1024-key sequence of its batch. To keep the program
SPMD-static, the host rolls each core's x rows (and noise columns / mask)
so the query rows are always tokens 0..511.

Math folding used on-device:
    logits = qk/8 + 5*noise + mask
           = 5*((q/40)@k^T + noise) + mask
    Wq,bq are host-prescaled by 1/40 so the scores matmul directly yields
    qk/40.  exp(mask) is folded into V' rows (and the fused sum-of-exp
    column), which is exactly softmax with the additive key mask.
"""

import os
import threading

import numpy as np

import concourse.bass as bass
import concourse.mybir as mybir
import concourse.tile as tile
from concourse import bacc
from concourse.masks import make_identity

F32 = mybir.dt.float32
F32R = mybir.dt.float32r
F16 = mybir.dt.float16

B, S, D, H, HD = 4, 1024, 768, 12, 64
NCORES = 8
SQ = 512            # query rows per core
QT = SQ // 128      # 4 query tiles
KT = S // 128       # 8 key token tiles
FC = D // 128       # 6 feature chunks
NPAIR = H // 2      # 6 head pairs
EXP_SCALE = 5.0
EXP_BIAS = -25.0
EPS1, EPS2 = 1e-12, 1e-5

Identity = mybir.ActivationFunctionType.Identity
Copy = mybir.ActivationFunctionType.Copy
Exp = mybir.ActivationFunctionType.Exp
Sqrt = mybir.ActivationFunctionType.Sqrt
ADD = mybir.AluOpType.add
SUB = mybir.AluOpType.subtract
MUL = mybir.AluOpType.mult


def build_program():
    nc = bacc.Bacc(trn_type="TRN2", num_devices=NCORES)

    x = nc.dram_tensor("x", [S, D], F32, kind="ExternalInput")
    noise = nc.dram_tensor("noise", [H, SQ, S], F32, kind="ExternalInput")
    mask = nc.dram_tensor("mask", [S], F32, kind="ExternalInput")
    wq = nc.dram_tensor("wq", [D, D], F32, kind="ExternalInput")
    wk = nc.dram_tensor("wk", [D, D], F32, kind="ExternalInput")
    wv = nc.dram_tensor("wv", [D, D], F32, kind="ExternalInput")
    wd = nc.dram_tensor("wd", [D, D], F32, kind="ExternalInput")
    bq = nc.dram_tensor("bq", [D], F32, kind="ExternalInput")
    bk = nc.dram_tensor("bk", [D], F32, kind="ExternalInput")
    bv = nc.dram_tensor("bv", [D], F32, kind="ExternalInput")
    bd = nc.dram_tensor("bd", [D], F32, kind="ExternalInput")
    g1 = nc.dram_tensor("g1", [D], F32, kind="ExternalInput")
    b1 = nc.dram_tensor("b1", [D], F32, kind="ExternalInput")
    g2 = nc.dram_tensor("g2", [D], F32, kind="ExternalInput")
    b2 = nc.dram_tensor("b2", [D], F32, kind="ExternalInput")
    out = nc.dram_tensor("out", [SQ, D], F32, kind="ExternalOutput")

    with tile.TileContext(nc) as tc:
        emit(nc, tc, x, noise, mask, wq, wk, wv, wd,
             bq, bk, bv, bd, g1, b1, g2, b2, out)
    nc.finalize()
    return nc


def emit(nc, tc, x, noise, mask, wq, wk, wv, wd,
         bq, bk, bv, bd, g1, b1, g2, b2, out):
    from contextlib import ExitStack

    ctx = ExitStack()
    with ctx:
        const = ctx.enter_context(tc.tile_pool(name="const", bufs=1))

        # ---- constants / broadcasts -------------------------------------
        ident = const.tile([128, 128], F32)
        make_identity(nc, ident)

        cexpb = const.tile([128, 1], F32)   # exp bias
        nc.vector.memset(cexpb, EXP_BIAS)
        ceps1 = const.tile([128, 1], F32)
        nc.vector.memset(ceps1, EPS1)
        ceps2 = const.tile([128, 1], F32)
        nc.vector.memset(ceps2, EPS2)
        eps_tiles = {EPS1: ceps1, EPS2: ceps2}

        bqc = const.tile([128, FC], F32)   # bq (prescaled) as per-partition cols
        bkc = const.tile([128, FC], F32)
        nc.gpsimd.dma_start(out=bqc, in_=bq.ap().rearrange("(c p) -> p c", p=128))
        nc.gpsimd.dma_start(out=bkc, in_=bk.ap().rearrange("(c p) -> p c", p=128))

        mk = const.tile([128, KT], F32)
        nc.gpsimd.dma_start(out=mk, in_=mask.ap().rearrange("(t p) -> p t", p=128))
        emk = const.tile([128, KT], F32)
        nc.scalar.activation(emk, mk, Exp)

        bvb = const.tile([128, D], F32)    # row broadcasts across partitions
        g1b = const.tile([128, D], F32)
        b1b = const.tile([128, D], F32)
        g2b = const.tile([128, D], F32)
        b2b = const.tile([128, D], F32)
        bdb = const.tile([128, D], F32)
        for dst, src in ((bvb, bv), (g1b, g1), (b1b, b1),
                         (g2b, g2), (b2b, b2), (bdb, bd)):
            nc.gpsimd.dma_start(out=dst, in_=src.ap().partition_broadcast(128))

        # persistent outputs of the projection phase
        pqkv = ctx.enter_context(tc.tile_pool(name="pqkv", bufs=1))
        qts = pqkv.tile([128, FC, SQ], F16)   # q^T/40 feature-major
        kts = pqkv.tile([128, FC, S], F16)    # k^T feature-major
        vaug = pqkv.tile([128, KT, H, 65], F16)  # [(v+bv)*emk | emk] token-major
        bdx = const.tile([128, QT, D], F32)   # bd + x  (dense bias + residual)
        b1x = const.tile([128, QT, D], F32)   # b1 + x  (LN1 bias + residual)

        # ---- phase A/B: load x, x^T, projections (transient pools) ------
        ab = ExitStack()
        px = ab.enter_context(tc.tile_pool(name="px", bufs=1))
        pxt = ab.enter_context(tc.tile_pool(name="pxt", bufs=1))
        pw = ab.enter_context(tc.tile_pool(name="pw", bufs=FC + 3))
        pproj = ab.enter_context(tc.tile_pool(name="pproj", bufs=3, space="PSUM"))
        pvtmp = ab.enter_context(tc.tile_pool(name="pvtmp", bufs=3))

        xsb = px.tile([128, KT, D], F32)   # xsb[p, t, d] = x[t*128+p, d]
        nc.scalar.dma_start(out=xsb, in_=x.ap().rearrange("(t p) d -> p t d", p=128))

        for t in range(QT):
            nc.vector.tensor_tensor(bdx[:, t, :], xsb[:, t, :], bdb, ADD)
            nc.vector.tensor_tensor(b1x[:, t, :], xsb[:, t, :], b1b, ADD)

        xt = pxt.tile([128, FC, S], F16)   # xt[p, c, t] = x[t, c*128+p]
        with tc.tile_pool(name="pstr", bufs=4, space="PSUM") as pstr:
            for c in range(FC):
                for t in range(KT):
                    ps = pstr.tile([128, 128], F32, name=f"tr_{c}_{t}", tag="tr")
                    nc.tensor.transpose(ps, xsb[:, t, c * 128:(c + 1) * 128], ident)
                    nc.scalar.copy(xt[:, c, t * 128:(t + 1) * 128], ps)

        for wsrc, dst, bias_c, ntok in ((wq, qts, bqc, SQ), (wk, kts, bkc, S)):
            wt = [pw.tile([128, D], F16, name=f"w_{i}", tag="w") for i in range(FC)]
            for i in range(FC):
                nc.gpsimd.dma_start(out=wt[i], in_=wsrc.ap()[i * 128:(i + 1) * 128, :])
            for fo in range(FC):
                for th in range(ntok // 512):
                    ps = pproj.tile([128, 512], F32, name=f"pp_{fo}_{th}", tag="pp")
                    for kc in range(FC):
                        nc.tensor.matmul(
                            ps, wt[kc][:, fo * 128:(fo + 1) * 128],
                            xt[:, kc, th * 512:(th + 1) * 512],
                            start=(kc == 0), stop=(kc == FC - 1))
                    nc.scalar.activation(dst[:, fo, th * 512:(th + 1) * 512], ps,
                                         Identity, bias=bias_c[:, fo:fo + 1])

        # V token-major, fused bias + mask-exp scale, cast to fp16
        wt = [pw.tile([128, D], F16, name=f"wv_{i}", tag="w") for i in range(FC)]
        for i in range(FC):
            nc.gpsimd.dma_start(out=wt[i], in_=wv.ap()[i * 128:(i + 1) * 128, :])
        for t in range(KT):
            for f0, fw in ((0, 512), (512, 256)):
                ps = pproj.tile([128, 512], F32, name=f"pv_{t}_{f0}", tag="pp")
                for kc in range(FC):
                    nc.tensor.matmul(
                        ps[:, :fw], xt[:, kc, t * 128:(t + 1) * 128],
                        wt[kc][:, f0:f0 + fw],
                        start=(kc == 0), stop=(kc == FC - 1))
                vt = pvtmp.tile([128, 512], F32, name=f"vt_{t}_{f0}", tag="vt")
                nc.vector.tensor_tensor(vt[:, :fw], ps[:, :fw], bvb[:, f0:f0 + fw], ADD)
                nc.vector.tensor_scalar(
                    out=vaug[:, t, f0 // 64:(f0 + fw) // 64, 0:64],
                    in0=vt[:, :fw].rearrange("p (h d) -> p h d", d=64),
                    scalar1=emk[:, t:t + 1], scalar2=None,
                    op0=MUL)
        for h in range(H):
            nc.scalar.copy(vaug[:, :, h, 64], emk)
        ab.close()

        # ---- attention ---------------------------------------------------
        pct = ctx.enter_context(tc.tile_pool(name="pct", bufs=1))
        cts = pct.tile([128, NPAIR, SQ], F16)   # normalized ctx^T, head pairs

        att = ExitStack()
        patt = att.enter_context(tc.tile_pool(name="patt", bufs=3))
        ppt = att.enter_context(tc.tile_pool(name="ppt", bufs=2))
        psc = att.enter_context(tc.tile_pool(name="psc", bufs=1, space="PSUM"))
        psct = att.enter_context(tc.tile_pool(name="psct", bufs=4, space="PSUM"))
        pdram = att.enter_context(tc.tile_pool(name="pdram", bufs=3, space="DRAM"))
        pnrm = att.enter_context(tc.tile_pool(name="pnrm", bufs=2))

        for hp in range(NPAIR):
            pt = [ppt.tile([128, KT, SQ], F16, name=f"pt{i}_{hp}", tag=f"pt{i}")
                  for i in range(2)]
            for qt in range(QT):
                pss = [psc.tile([128, S], F32, name=f"ps_{hp}_{qt}_{i}", tag=f"ps{i}")
                       for i in range(2)]
                qsl = slice(qt * 128, (qt + 1) * 128)
                # 2-head row-packed fp32r scores: psum = (q/40) @ k^T
                for kh in range(2):
                    ksl = slice(kh * 512, (kh + 1) * 512)
                    for i in range(2):
                        hsl = slice(i * 64, (i + 1) * 64)
                        nc.tensor.matmul(
                            pss[i][:, ksl], qts[hsl, hp, qsl],
                            kts[hsl, hp, ksl],
                            start=True, stop=True, tile_position=(i * 64, 0))
                for i in range(2):
                    h = 2 * hp + i
                    nt = patt.tile([128, S], F32, name=f"nt_{hp}_{qt}_{i}", tag="nt")
                    neng = nc.gpsimd if (qt + i) % 2 == 0 else nc.scalar
                    neng.dma_start(out=nt, in_=noise.ap()[h, qsl, :])
                    et = patt.tile([128, S], F16, name=f"et_{hp}_{qt}_{i}", tag="et")
                    for kh in range(2):
                        ksl = slice(kh * 512, (kh + 1) * 512)
                        lt = patt.tile([128, 512], F32,
                                       name=f"lt_{hp}_{qt}_{i}_{kh}", tag="lt")
                        nc.vector.tensor_tensor(lt, pss[i][:, ksl], nt[:, ksl], ADD)
                        nc.scalar.activation(et[:, ksl], lt, Exp,
                                             bias=cexpb, scale=EXP_SCALE)
                    teng = nc.sync if (qt + i) % 2 == 0 else nc.scalar
                    teng.dma_start(out=pt[i][:, :, qsl], in_=et, transpose=True)

            # ctx per head: psum rows 0:64 = V'^T @ P^T, row 64 = sum-of-exp
            for i in range(2):
                h = 2 * hp + i
                pc = psct.tile([65, 512], F32, name=f"pc_{hp}_{i}", tag="pc")
                for t in range(KT):
                    nc.tensor.matmul(pc, vaug[:, t, h, 0:65], pt[i][:, t, :],
                                     start=(t == 0), stop=(t == KT - 1))
                # reciprocal of sumexp row, broadcast to 64 partitions via DMA
                rt = pnrm.tile([65, 512], F32, name=f"rt_{hp}_{i}", tag="rt")
                nc.vector.reciprocal(rt[64:65, :], pc[64:65, :])
                sc = pdram.tile([1, 512], F32, name=f"sc_{hp}_{i}", tag="sc")
                nc.gpsimd.dma_start(out=sc, in_=rt[64:65, :])
                rb = pnrm.tile([64, 512], F32, name=f"rb_{hp}_{i}", tag="rb")
                nc.gpsimd.dma_start(out=rb, in_=sc[0, :].partition_broadcast(64))
                if i == 0:
                    nc.vector.tensor_tensor(cts[0:64, hp, :], pc[0:64, :], rb, MUL)
                else:
                    co = pnrm.tile([64, 512], F32, name=f"co_{hp}", tag="co")
                    nc.vector.tensor_tensor(co, pc[0:64, :], rb, MUL)
                    # partition shift 0:64 -> 64:128 (only DMA can do this)
                    nc.gpsimd.dma_start(out=cts[64:128, hp, :], in_=co)
        att.close()

        # ---- dense + residual + two LayerNorms --------------------------
        pwd = ctx.enter_context(tc.tile_pool(name="pwd", bufs=1))
        wdt = [pwd.tile([128, D], F16, name=f"wd_{i}", tag=f"wd_{i}") for i in range(FC)]
        for i in range(FC):
            nc.gpsimd.dma_start(out=wdt[i], in_=wd.ap()[i * 128:(i + 1) * 128, :])

        pd = ctx.enter_context(tc.tile_pool(name="pd", bufs=3))
        psd = ctx.enter_context(tc.tile_pool(name="psd", bufs=3, space="PSUM"))
        pst = ctx.enter_context(tc.tile_pool(name="pst", bufs=4))

        for t in range(QT):
            qsl = slice(t * 128, (t + 1) * 128)
            t1 = pd.tile([128, D], F32, name=f"t1_{t}", tag="t1")
            for f0, fw in ((0, 512), (512, 256)):
                ps = psd.tile([128, 512], F32, name=f"pd_{t}_{f0}", tag="pd")
                for dc in range(FC):
                    nc.tensor.matmul(ps[:, :fw], cts[:, dc, qsl],
                                     wdt[dc][:, f0:f0 + fw],
                                     start=(dc == 0), stop=(dc == FC - 1))
                nc.vector.tensor_tensor(t1[:, f0:f0 + fw], ps[:, :fw],
                                        bdx[:, t, f0:f0 + fw], ADD)

            u = pd.tile([128, D], F32, name=f"u_{t}", tag="u")
            layernorm(nc, pst, t1, u, eps_tiles[EPS1], t)
            nc.vector.tensor_tensor(u, u, g1b, MUL)
            nc.vector.tensor_tensor(u, u, b1x[:, t, :], ADD)

            v = pd.tile([128, D], F32, name=f"v_{t}", tag="v")
            layernorm(nc, pst, u, v, eps_tiles[EPS2], t + 4)
            nc.vector.tensor_tensor(v, v, g2b, MUL)
            nc.vector.tensor_tensor(v, v, b2b, ADD)
            nc.gpsimd.dma_start(out=out.ap()[qsl, :], in_=v)


def layernorm(nc, pool, src, dst, eps_tile, uid):
    """dst = (src - mean) * rsqrt(var + eps), stats along the 768 free dim."""
    st = pool.tile([128, 3, 6], F32, name=f"st_{uid}", tag="st")
    for sg in range(3):
        nc.vector.bn_stats(st[:, sg, :], src[:, sg * 256:(sg + 1) * 256])
    mv = pool.tile([128, 2], F32, name=f"mv_{uid}", tag="mv")
    nc.vector.bn_aggr(mv, st)
    sd = pool.tile([128, 1], F32, name=f"sd_{uid}", tag="sd")
    nc.scalar.activation(sd, mv[:, 1:2], Sqrt, bias=eps_tile)
    rs = pool.tile([128, 1], F32, name=f"rs_{uid}", tag="rs")
    nc.vector.reciprocal(rs, sd)
    nc.vector.tensor_scalar(out=dst, in0=src, scalar1=mv[:, 0:1], scalar2=rs,
                            op0=SUB, op1=MUL)


_lock = threading.Lock()
_cache = {}


def get_program():
    with _lock:
        if "nc" not in _cache:
            _cache["nc"] = build_program()
        return _cache["nc"]


def make_in_maps(inputs):
    x = np.ascontiguousarray(np.asarray(inputs["input_tensor"], dtype=np.float32))
    am = np.asarray(inputs["attention_mask"], dtype=np.float32)
    nz = np.asarray(inputs["noise"], dtype=np.float32)
    shared = {
        "wq": np.ascontiguousarray(np.asarray(inputs["Wq"], np.float32) / 40.0),
        "bq": np.ascontiguousarray(np.asarray(inputs["bq"], np.float32) / 40.0),
        "wk": np.ascontiguousarray(np.asarray(inputs["Wk"], np.float32)),
        "bk": np.ascontiguousarray(np.asarray(inputs["bk"], np.float32)),
        "wv": np.ascontiguousarray(np.asarray(inputs["Wv"], np.float32)),
        "bv": np.ascontiguousarray(np.asarray(inputs["bv"], np.float32)),
        "wd": np.ascontiguousarray(np.asarray(inputs["Wd"], np.float32)),
        "bd": np.ascontiguousarray(np.asarray(inputs["bd"], np.float32)),
        "g1": np.ascontiguousarray(np.asarray(inputs["g1"], np.float32)),
        "b1": np.ascontiguousarray(np.asarray(inputs["b1"], np.float32)),
        "g2": np.ascontiguousarray(np.asarray(inputs["g2"], np.float32)),
        "b2": np.ascontiguousarray(np.asarray(inputs["b2"], np.float32)),
    }
    in_maps = []
    for c in range(NCORES):
        b, half = c // 2, c % 2
        qoff = half * SQ
        m = dict(shared)
        m["x"] = np.ascontiguousarray(np.roll(x[b], -qoff, axis=0))
        m["noise"] = np.ascontiguousarray(
            np.roll(nz[b, :, qoff:qoff + SQ, :], -qoff, axis=-1))
        m["mask"] = np.ascontiguousarray(np.roll(am[b, 0, 0], -qoff))
        in_maps.append(m)
    return in_maps


def kernel(**inputs):
    from concourse.bass_utils import run_bass_kernel_spmd

    nc = get_program()
    in_maps = make_in_maps(inputs)
    res = run_bass_kernel_spmd(
        nc, in_maps, core_ids=list(range(NCORES)),
        trace=bool(int(os.environ.get("KERNEL_TRACE", "0"))))
    kernel.last_results = res
    outp = np.empty((B, S, D), dtype=np.float32)
    for c in range(NCORES):
        b, half = c // 2, c % 2
        outp[b, half * SQ:(half + 1) * SQ, :] = res.results[c]["out"]
    return outp
